# revision 24
# baseline (speedup 1.0000x reference)
"""LiquidS4Cell kernel for 8x Trainium2 NeuronCores (batch-parallel).

y[b] = scan(u[b]) @ (dt*B*Q), where scan is the diagonal recurrence
x_t = a * x_{t-1} + u_t  (a = exp(-dt*exp(Lambda)), Bdt folded into Q).

Per-core plan (one batch element per core):
  - DMA u chunk (512 rows) in natural (l, n) layout
  - PE transpose 128x128 blocks -> PSUM in (n, l) layout
  - DVE tensor_tensor_scan along free dim (exact fp32 recurrence)
  - PE matmul xsT @ Qs (float32r) accumulating over n-tiles
  - ACT copies PSUM y -> SBUF, DMA out
"""

import os
import sys

sys.path.insert(0, "/opt/trn_rl_repo")

import numpy as np

L, N, D = 4096, 512, 512
P = 128
NT = N // P            # 4 n-tiles
CHUNK = 512
NCH = L // CHUNK       # 8 chunks
SUB = CHUNK // P       # 4 l-subtiles per chunk

MM_DT = os.environ.get("LS4_MM_DT", "float32r")

_cache = {}
_IDENT = np.eye(128, dtype=np.float32)


def _split_excess_waits(nc, mybir, max_waits=1):
    """walrus codegen on this container only accepts 1 sync-wait per
    instruction; split extras into preceding same-engine drains."""
    n = 0
    for f in nc.m.functions:
        for b in f.blocks:
            out = []
            for i in b.instructions:
                si = i.sync_info
                if si is not None and len(si.on_wait) > max_waits:
                    waits = list(si.on_wait)
                    extra, keep = waits[:-max_waits], waits[-max_waits:]
                    for ci in range(0, len(extra), max_waits):
                        d = mybir.InstDrain(
                            name=f"{i.name}-ws{ci}", engine=i.engine, ins=[], outs=[]
                        )
                        d.sync_info = mybir.SyncInfo(
                            on_wait=extra[ci : ci + max_waits], on_update=[]
                        )
                        out.append(d)
                        n += 1
                    i.sync_info = mybir.SyncInfo(
                        on_wait=keep, on_update=list(si.on_update)
                    )
                out.append(i)
            b.instructions[:] = out
    return n


def _build_nc():
    import concourse.bass as bass
    import concourse.mybir as mybir
    import concourse.tile as tile

    f32 = mybir.dt.float32
    mmdt = getattr(mybir.dt, MM_DT)
    # f32r shares numpy float32; qs dram can be declared mmdt directly for
    # f32r/f32. bf16 path casts during SWDGE DMA.
    qs_dram_dt = mmdt if MM_DT != "bfloat16" else f32
    u_dt = mmdt if MM_DT == "float32r" else f32

    nc = bass.Bass()
    u = nc.dram_tensor("u", [L, N], u_dt, kind="ExternalInput")
    ident_d = nc.dram_tensor("ident", [P, P], u_dt, kind="ExternalInput")
    arep = nc.dram_tensor("arep", [N, 1], f32, kind="ExternalInput")
    qs = nc.dram_tensor("qs", [N, D], qs_dram_dt, kind="ExternalInput")
    y = nc.dram_tensor("y", [L, D], f32, kind="ExternalOutput")

    with tile.TileContext(nc) as tc:
        with (
            tc.tile_pool(name="const", bufs=1) as constp,
            tc.tile_pool(name="xst", bufs=1) as xstp,
            tc.tile_pool(name="io", bufs=4) as iop,
            tc.tile_pool(name="psu", bufs=1, space="PSUM") as psu,
            tc.tile_pool(name="psy", bufs=1, space="PSUM") as psy,
        ):
            ident = constp.tile([P, P], u_dt, tag="ident")
            nc.scalar.dma_start(out=ident[:], in_=bass.AP(ident_d, 0, [[P, P], [1, P]]))

            a_sb, q_sb = [], []
            zeros = constp.tile([P, CHUNK], f32, tag="zeros")
            nc.gpsimd.memset(zeros[:], 0.0)
            acol = constp.tile([P, NT], f32, tag="acol")
            nc.scalar.dma_start(
                out=acol[:], in_=bass.AP(arep, 0, [[1, P], [P, NT]])
            )
            for t in range(NT):
                at = constp.tile([P, CHUNK], f32, tag=f"a{t}")
                nc.scalar.activation(
                    at[:], zeros[:], mybir.ActivationFunctionType.Identity,
                    bias=acol[:, t : t + 1],
                )
                a_sb.append(at)
                qt = constp.tile([P, D], mmdt, tag=f"q{t}")
                qdma = nc.scalar if qs_dram_dt == mmdt else nc.gpsimd
                qdma.dma_start(
                    out=qt[:], in_=bass.AP(qs, t * P * D, [[D, P], [1, D]])
                )
                q_sb.append(qt)

            xsT = [xstp.tile([P, L], mmdt, tag=f"xsT{t}", name=f"xsT{t}") for t in range(NT)]
            uT = [psu.tile([P, CHUNK], u_dt, tag=f"uT{t}", name=f"uT{t}") for t in range(NT)]
            yps = [psy.tile([P, D], f32, tag=f"y{s}", name=f"y{s}") for s in range(SUB)]

            IOB = 1  # chunks per IO batch
            for kb in range(NCH // IOB):
                u_sb = iop.tile([P, IOB * SUB * N], u_dt, tag="u_sb", name=f"u_sb{kb}")
                if kb < 2:
                    # fine grain during pipeline fill: compute starts after 256KB
                    for si in range(IOB * SUB):
                        nc.sync.dma_start(
                            out=u_sb[:, si * N : (si + 1) * N],
                            in_=bass.AP(
                                u,
                                (kb * IOB * SUB + si) * P * N,
                                [[N, P], [1, N]],
                            ),
                        )
                else:
                    # steady state: one 1MB transfer, fewer fixed costs
                    nc.sync.dma_start(
                        out=u_sb[:].rearrange("p (c n) -> p c n", n=N),
                        in_=bass.AP(
                            u,
                            kb * IOB * CHUNK * N,
                            [[N, P], [P * N, IOB * SUB], [1, N]],
                        ),
                    )
                y_sb = iop.tile([P, IOB * SUB * D], f32, tag="y_sb", name=f"y_sb{kb}")
                for ki in range(IOB):
                    k = kb * IOB + ki
                    # transpose blocks into PSUM (n, l) layout
                    for t in range(NT):
                        for s in range(SUB):
                            nc.tensor.transpose(
                                uT[t][:, s * P : (s + 1) * P],
                                u_sb[
                                    :,
                                    (ki * SUB + s) * N + t * P : (ki * SUB + s) * N
                                    + (t + 1) * P,
                                ],
                                ident[:],
                            )
                    # scan along l (chained via initial from previous chunk)
                    for t in range(NT):
                        nc.vector.tensor_tensor_scan(
                            xsT[t][:, k * CHUNK : (k + 1) * CHUNK],
                            a_sb[t][:],
                            uT[t][:],
                            0.0 if k == 0 else xsT[t][:, k * CHUNK - 1 : k * CHUNK],
                            mybir.AluOpType.mult,
                            mybir.AluOpType.add,
                        )
                    # project: y[l, d] = sum_n xsT[n, l] * Qs[n, d]
                    for s in range(SUB):
                        for t in range(NT):
                            nc.tensor.matmul(
                                yps[s][:],
                                xsT[t][
                                    :, k * CHUNK + s * P : k * CHUNK + (s + 1) * P
                                ],
                                q_sb[t][:],
                                start=(t == 0),
                                stop=(t == NT - 1),
                            )
                        nc.scalar.copy(
                            y_sb[:, (ki * SUB + s) * D : (ki * SUB + s + 1) * D],
                            yps[s][:],
                        )
                nc.scalar.dma_start(
                    out=bass.AP(
                        y,
                        kb * IOB * CHUNK * D,
                        [[D, P], [P * D, IOB * SUB], [1, D]],
                    ),
                    in_=y_sb[:].rearrange("p (c d) -> p c d", d=D),
                )
    _split_excess_waits(nc, mybir)
    return nc


def _get_nc():
    key = MM_DT
    if key not in _cache:
        _cache[key] = _build_nc()
    return _cache[key]


def _prep_params(Lambda, B, Q, log_dt):
    dt = np.exp(log_dt.astype(np.float32))[0]
    a = np.exp(dt * -np.exp(Lambda.astype(np.float32))).astype(np.float32)
    arep = np.ascontiguousarray(a[:, None])
    qsm = np.ascontiguousarray((dt * B.astype(np.float32))[:, None] * Q.astype(np.float32)).astype(np.float32)
    return arep, qsm


def _run(u, Lambda, B, Q, log_dt, trace=False):
    from concourse.bass_utils import run_bass_kernel_spmd

    nc = _get_nc()
    u = np.asarray(u, dtype=np.float32)
    arep, qsm = _prep_params(
        np.asarray(Lambda), np.asarray(B), np.asarray(Q), np.asarray(log_dt)
    )
    batch = u.shape[0]
    in_maps = [
        {"u": np.ascontiguousarray(u[b]), "arep": arep, "qs": qsm,
         "ident": _IDENT}
        for b in range(batch)
    ]
    res = run_bass_kernel_spmd(
        nc, in_maps, core_ids=list(range(batch)), trace=trace
    )
    out = np.stack([res.results[b]["y"] for b in range(batch)], axis=0)
    return out, res


def kernel(u, Lambda, B, Q, log_dt):
    out, _ = _run(u, Lambda, B, Q, log_dt, trace=False)
    return out
